# revision 15
# baseline (speedup 1.0000x reference)
"""Trainium2 Bass kernel for nn_DistLayer (segment-mean pooling + fc + BatchNorm + ReLU).

Contract: kernel(**inputs) takes FULL unsharded numpy inputs and returns the
FULL [131072, 256] float32 output. Internally shards rows across 8 NeuronCores.

Math (reference):
    pooled_atom = segment_mean(x[:, :128], atom_idx)[atom_idx]
    pooled_ele  = segment_mean(x[:, 128:256], atom_idx)[ele_idx]
    h = concat([x_atom, pooled_atom, x_ele, pooled_ele, x_dist]) @ W1 + b1
    out = relu(batchnorm(h))                    (training-mode batch stats)

Device decomposition (per core, h kept feature-major "h^T" [256, rows]):
    h^T = Wx^T@x^T + W_pa^T@Ma[atom_idx]^T + W_pe^T@Me[ele_idx]^T   (+b1 cancels in BN)
    Ma/Me = global segment means, computed via per-block one-hot matmuls then a
    scatter-accumulate into a DRAM table, AllReduce'd across the 8 cores.
    BN stats accumulated per-partition (feature) during the PSUM->SBUF flush,
    AllReduce'd, and applied with a single fused scale+bias+relu pass.
"""

import os
from contextlib import ExitStack

import numpy as np

import concourse.bass as bass
import concourse.tile as tile
from concourse import bacc, mybir
from concourse.bass_utils import run_bass_kernel_spmd

LAST_NC = None  # most recent built program (for cost-model timing in test.py)
DISABLE = set(filter(None, os.environ.get("K_DISABLE", "").split(",")))

F32 = mybir.dt.float32
F32R = mybir.dt.float32r
F16 = mybir.dt.float16
I16 = mybir.dt.int16
I32 = mybir.dt.int32

N_AE = 128
N_DE = 128
NUM_SEG = 4096
EPS = 1e-5
D_IN = 384            # x feature dim
D_OUT = 256           # output feature dim
BLK = 512             # rows per block
TPB = BLK // 128      # row-tiles per block


def _wrap_idx16(idx):
    """dma_gather index layout: idx i at [i%16, i//16], replicated to 128 partitions."""
    n = idx.shape[0]
    w = idx.reshape(n // 16, 16).T.astype(np.int16)   # [16, n/16]
    return np.tile(w, (8, 1))                          # [128, n/16]


def build_program(n_cores, rpc, w_blk, num_seg=NUM_SEG, gsz=1):
    """Build the (core-uniform) bass program. rpc = rows per core."""
    nblk = rpc // BLK
    nc = bacc.Bacc("TRN2", target_bir_lowering=False, debug=False,
                   num_devices=n_cores)

    # ---- I/O tensors (per-core) ----
    d_xt = nc.dram_tensor("xt", [D_IN, rpc], F32R, kind="ExternalInput").ap()
    d_xae = nc.dram_tensor("xae", [rpc, 2 * N_AE], F16, kind="ExternalInput").ap()
    NSEG = num_seg
    d_oh = nc.dram_tensor("oh", [nblk, 128, TPB * w_blk], F16, kind="ExternalInput").ap()
    d_scl = nc.dram_tensor("scl", [w_blk, nblk // gsz], F32, kind="ExternalInput").ap()
    d_offs = nc.dram_tensor("offs", [w_blk, nblk // gsz], I32, kind="ExternalInput").ap()
    d_ga = nc.dram_tensor("ga", [nblk, 128, BLK // 16], I16, kind="ExternalInput").ap()
    d_ge = nc.dram_tensor("ge", [nblk, 128, BLK // 16], I16, kind="ExternalInput").ap()
    d_wx = nc.dram_tensor("wx", [D_IN, D_OUT], F32, kind="ExternalInput").ap()
    d_wpa = nc.dram_tensor("wpa", [N_AE, D_OUT], F16, kind="ExternalInput").ap()
    d_wpe = nc.dram_tensor("wpe", [N_AE, D_OUT], F16, kind="ExternalInput").ap()
    d_gb = nc.dram_tensor("gb", [128, 4], F32, kind="ExternalInput").ap()

    d_out = nc.dram_tensor("out", [D_OUT, rpc], F32, kind="ExternalOutput").ap()

    groups = [list(range(n_cores))]

    with tile.TileContext(nc) as tc, ExitStack() as ctx:
        const = ctx.enter_context(tc.tile_pool(name="const", bufs=1))
        store = ctx.enter_context(tc.tile_pool(name="store", bufs=1))
        strm = ctx.enter_context(tc.tile_pool(name="strm", bufs=3))
        ps = ctx.enter_context(tc.tile_pool(name="ps", bufs=2, space="PSUM"))
        dram = ctx.enter_context(tc.tile_pool(name="dram", bufs=1, space="DRAM"))

        # internal DRAM
        TAB = NSEG + 128   # +128 dummy rows absorb out-of-span scatter lanes
        ftable = dram.tile([TAB, D_OUT], F32)           # f32 partial mean table
        ptable = dram.tile([TAB, D_OUT], F16)           # f16 cast for allreduce
        gtable = dram.tile([TAB, D_OUT], F16, addr_space="Shared")
        statin = dram.tile([128, 4], F32)
        statout = dram.tile([128, 4], F32, addr_space="Shared")

        # ---- constants in SBUF ----
        wxr = const.tile([128, 3 * D_OUT], F32R)
        nc.gpsimd.dma_start(wxr[:].rearrange("p (c f) -> p c f", c=3),
                            d_wx.rearrange("(c p) f -> p c f", p=128))
        wpa = const.tile([128, D_OUT], F16)
        nc.sync.dma_start(wpa[:], d_wpa[:])
        wpe = const.tile([128, D_OUT], F16)
        nc.sync.dma_start(wpe[:], d_wpe[:])
        scl = const.tile([w_blk, nblk // gsz], F32)
        nc.sync.dma_start(scl[:], d_scl[:])
        offs = const.tile([w_blk, nblk // gsz], I32)
        nc.sync.dma_start(offs[:], d_offs[:])
        gb = const.tile([128, 4], F32)
        nc.sync.dma_start(gb[:], d_gb[:])

        # persistent h^T store: 2 chunks of [128, rpc]
        hsb = [store.tile([128, rpc], F32, name=f"hsb{m}", tag=f"hsb{m}")
               for m in range(2)]
        sums = store.tile([128, 4 * nblk], F32)   # [sh0 | sh1 | shh0 | shh1]

        # ---- zero the f32 table ----
        nzc = TAB // 128  # table column-chunks
        zch = min(nzc, 4)
        zt = const.tile([128, zch * D_OUT], F32)
        nc.vector.memset(zt[:], 0.0)
        ftab_v = ftable[:].rearrange("(c p) f -> p c f", p=128)  # [128, nzc, 256]
        for i0 in range(0, nzc, zch):
            w = min(zch, nzc - i0)
            nc.sync.dma_start(ftab_v[:, i0:i0 + w, :],
                              zt[:, :w * D_OUT].rearrange("p (c f) -> p c f", c=w))

        # ---- P1: segment sums -> scatter-accumulate into ftable ----
        ngrp = nblk // gsz
        for g in range(ngrp) if "p1" not in DISABLE else []:
            seg = ps.tile([w_blk, D_OUT], F32, name="seg", tag="seg")
            for j in range(gsz):
                b = g * gsz + j
                xae = strm.tile([128, TPB * 2 * N_AE], F16, name="xae", tag="xae")
                nc.sync.dma_start(
                    xae[:].rearrange("p (q f) -> p q f", q=TPB),
                    d_xae.rearrange("(q p) f -> p q f",
                                    p=128)[:, TPB * b:TPB * (b + 1), :])
                oh = strm.tile([128, TPB * w_blk], F16, name="oh", tag="oh")
                nc.sync.dma_start(oh[:], d_oh[b])
                for t in range(TPB):
                    nc.tensor.matmul(seg[:],
                                     oh[:, w_blk * t:w_blk * (t + 1)],
                                     xae[:, 2 * N_AE * t:2 * N_AE * (t + 1)],
                                     start=(j == 0 and t == 0),
                                     stop=(j == gsz - 1 and t == TPB - 1))
            ssb = strm.tile([w_blk, D_OUT], F32, name="ssb", tag="ssb", bufs=2)
            # scale by 1/global_count while flushing PSUM -> SBUF
            nc.scalar.activation(ssb[:], seg[:],
                                 mybir.ActivationFunctionType.Identity,
                                 bias=0.0, scale=scl[:, g:g + 1])
            if "scatter" not in DISABLE:
                nc.gpsimd.indirect_dma_start(
                    out=ftable[:],
                    out_offset=bass.IndirectOffsetOnAxis(ap=offs[:, g:g + 1],
                                                         axis=0),
                    in_=ssb[:],
                    in_offset=None,
                    compute_op=mybir.AluOpType.add,
                )

        # ---- cast table to f16 and AllReduce across cores ----
        t16 = const.tile([128, nzc * D_OUT], F16, name="t16")
        nc.gpsimd.dma_start(t16[:].rearrange("p (c f) -> p c f", c=nzc),
                            ftab_v)
        nc.sync.dma_start(ptable[:].rearrange("(c p) f -> p c f", p=128),
                          t16[:].rearrange("p (c f) -> p c f", c=nzc))
        nc.gpsimd.collective_compute(
            "AllReduce", mybir.AluOpType.add, replica_groups=groups,
            ins=[ptable.opt()], outs=[gtable.opt()])

        # ---- P2: h^T = Wx^T x^T + W_pa^T Ma_g^T + W_pe^T Me_g^T ----
        for b in range(nblk):
            xtr = strm.tile([128, 3 * BLK], F32R, name="xtr", tag="xtr")
            nc.sync.dma_start(
                xtr[:].rearrange("p (c n) -> p c n", c=3),
                d_xt.rearrange("(c p) n -> p c n", p=128)[:, :, BLK * b:BLK * (b + 1)])
            gai = strm.tile([128, BLK // 16], I16, name="gai", tag="gai")
            nc.sync.dma_start(gai[:], d_ga[b])
            gei = strm.tile([128, BLK // 16], I16, name="gei", tag="gei")
            nc.sync.dma_start(gei[:], d_ge[b])
            gat = strm.tile([128, BLK], F16, name="gat", tag="gat")
            if "gather" in DISABLE:
                nc.sync.dma_start(gat[:].rearrange("p (c f) -> p c f", c=4),
                                  gtable[:].rearrange("(c p) f -> p c f",
                                                      p=128)[:, 0:4, 0:128])
            else:
                nc.gpsimd.dma_gather(
                    out_ap=gat[:].rearrange("p (a n) -> p a n", a=1),
                    in_ap=gtable[:, 0:N_AE],
                    idxs_ap=gai[:], num_idxs=BLK, num_idxs_reg=BLK,
                    elem_size=N_AE, elem_step=D_OUT, transpose=True)
            get = strm.tile([128, BLK], F16, name="get", tag="get")
            if "gather" in DISABLE:
                nc.sync.dma_start(get[:].rearrange("p (c f) -> p c f", c=4),
                                  gtable[:].rearrange("(c p) f -> p c f",
                                                      p=128)[:, 0:4, 0:128])
            else:
                nc.gpsimd.dma_gather(
                    out_ap=get[:].rearrange("p (a n) -> p a n", a=1),
                    in_ap=gtable[:, N_AE:2 * N_AE],
                    idxs_ap=gei[:], num_idxs=BLK, num_idxs_reg=BLK,
                    elem_size=N_AE, elem_step=D_OUT, transpose=True)

            for m in range(2):
                hp = ps.tile([128, BLK], F32, name=f"hp{m}", tag=f"hp{m}")
                for k in range(3):
                    nc.tensor.matmul(hp[:],
                                     wxr[:, D_OUT * k + 128 * m:
                                         D_OUT * k + 128 * (m + 1)],
                                     xtr[:, BLK * k:BLK * (k + 1)],
                                     start=(k == 0), stop=(k == 2))
                nc.tensor.matmul(hp[:], wpa[:, 128 * m:128 * (m + 1)], gat[:],
                                 start=False, stop=True, skip_group_check=True)
                nc.tensor.matmul(hp[:], wpe[:, 128 * m:128 * (m + 1)], get[:],
                                 start=False, stop=True, skip_group_check=True)
                # flush + per-feature sum
                nc.scalar.activation(hsb[m][:, BLK * b:BLK * (b + 1)], hp[:],
                                     mybir.ActivationFunctionType.Copy,
                                     accum_out=sums[:, nblk * m + b:
                                                    nblk * m + b + 1])
                dump = strm.tile([128, BLK], F32, name="dump", tag="dump", bufs=1)
                nc.scalar.activation(
                    dump[:], hsb[m][:, BLK * b:BLK * (b + 1)],
                    mybir.ActivationFunctionType.Square,
                    accum_out=sums[:, 2 * nblk + nblk * m + b:
                                   2 * nblk + nblk * m + b + 1])

        # ---- BN stats: reduce blocks, AllReduce cores, build scale/bias ----
        s4 = const.tile([128, 4], F32)
        for j in range(4):
            nc.vector.reduce_sum(s4[:, j:j + 1], sums[:, nblk * j:nblk * (j + 1)],
                                 axis=mybir.AxisListType.X)
        nc.sync.dma_start(statin[:], s4[:])
        nc.gpsimd.collective_compute(
            "AllReduce", mybir.AluOpType.add, replica_groups=groups,
            ins=[statin.opt()], outs=[statout.opt()])
        s4g = const.tile([128, 4], F32)
        nc.sync.dma_start(s4g[:], statout[:])

        n_total = float(n_cores * rpc)
        mu = const.tile([128, 2], F32)
        nc.scalar.mul(mu[:], s4g[:, 0:2], 1.0 / n_total)
        ex2 = const.tile([128, 2], F32)
        nc.scalar.mul(ex2[:], s4g[:, 2:4], 1.0 / n_total)
        mu2 = const.tile([128, 2], F32)
        nc.vector.tensor_tensor(out=mu2[:], in0=mu[:], in1=mu[:],
                                op=mybir.AluOpType.mult)
        nmu2 = const.tile([128, 2], F32)
        nc.scalar.mul(nmu2[:], mu2[:], -1.0)
        var = const.tile([128, 2], F32)
        nc.vector.tensor_tensor(out=var[:], in0=ex2[:], in1=nmu2[:],
                                op=mybir.AluOpType.add)
        vare = const.tile([128, 2], F32)
        nc.vector.tensor_scalar_add(vare[:], var[:], EPS)
        std = const.tile([128, 2], F32)
        nc.scalar.activation(std[:], vare[:], mybir.ActivationFunctionType.Sqrt,
                             bias=0.0)
        rstd = const.tile([128, 2], F32)
        nc.vector.reciprocal(rstd[:], std[:])
        # a = gamma * rstd ; baff = beta - mu * a
        a_t = const.tile([128, 2], F32)
        nc.vector.tensor_tensor(out=a_t[:], in0=gb[:, 0:2], in1=rstd[:],
                                op=mybir.AluOpType.mult)
        mua = const.tile([128, 2], F32)
        nc.vector.tensor_tensor(out=mua[:], in0=mu[:], in1=a_t[:],
                                op=mybir.AluOpType.mult)
        nmua = const.tile([128, 2], F32)
        nc.scalar.mul(nmua[:], mua[:], -1.0)
        baff = const.tile([128, 2], F32)
        nc.vector.tensor_tensor(out=baff[:], in0=gb[:, 2:4], in1=nmua[:],
                                op=mybir.AluOpType.add)

        # ---- P3: out = relu(h * a + b), streamed out ----
        for b in range(nblk):
            for m in range(2):
                osb = strm.tile([128, BLK], F32, name=f"osb{m}", tag=f"osb{m}")
                nc.scalar.activation(osb[:], hsb[m][:, BLK * b:BLK * (b + 1)],
                                     mybir.ActivationFunctionType.Relu,
                                     scale=a_t[:, m:m + 1], bias=baff[:, m:m + 1])
                nc.sync.dma_start(d_out[128 * m:128 * (m + 1),
                                        BLK * b:BLK * (b + 1)], osb[:])

    nc.compile()
    return nc


def prep_core_inputs(x, atom_idx, ele_idx, W1, gamma, beta, r0, r1, w_blk,
                     counts, num_seg=NUM_SEG, gsz=1):
    """Host-side shard prep. Pure slicing/layout/index work."""
    rpc = r1 - r0
    nblk = rpc // BLK
    xs = x[r0:r1]
    a = atom_idx[r0:r1]
    e = ele_idx[r0:r1]

    xt = np.ascontiguousarray(xs.T)                               # [384, rpc]
    xae = np.ascontiguousarray(xs[:, :2 * N_AE]).astype(np.float16)

    inv_cnt = (1.0 / np.maximum(counts, 1)).astype(np.float32)    # [4096]

    ngrp = nblk // gsz
    oh = np.zeros((nblk, 128, TPB * w_blk), dtype=np.float16)
    scl = np.zeros((w_blk, ngrp), dtype=np.float32)
    offs = np.zeros((w_blk, ngrp), dtype=np.int32)
    ga = np.zeros((nblk, 128, BLK // 16), dtype=np.int16)
    ge = np.zeros((nblk, 128, BLK // 16), dtype=np.int16)
    for g in range(ngrp):
        rows_g = a[BLK * gsz * g:BLK * gsz * (g + 1)]
        base = int(rows_g[0])
        span = int(rows_g[-1]) - base + 1
        assert span <= w_blk, f"group seg span {span} > w_blk {w_blk}"
        lanes = np.arange(w_blk)
        sidx = np.where(lanes < span, base + lanes, num_seg + (lanes % 128))
        scl[:, g] = inv_cnt[np.minimum(sidx, num_seg - 1)]
        offs[:, g] = sidx
        for j in range(gsz):
            b = g * gsz + j
            ab = a[BLK * b:BLK * (b + 1)]
            loc = (ab - base).astype(np.int64)
            for t in range(TPB):
                oh[b, np.arange(128), w_blk * t + loc[128 * t:128 * (t + 1)]] = 1.0
    for b in range(nblk):
        ga[b] = _wrap_idx16(a[BLK * b:BLK * (b + 1)])
        ge[b] = _wrap_idx16(e[BLK * b:BLK * (b + 1)])

    wx = np.ascontiguousarray(
        np.concatenate([W1[0:128], W1[256:384], W1[512:640]], axis=0))
    wpa = W1[128:256].astype(np.float16)
    wpe = W1[384:512].astype(np.float16)
    # gb layout: [:, 0:2] = gamma chunks, [:, 2:4] = beta chunks
    gbt = np.zeros((128, 4), dtype=np.float32)
    gbt[:, 0:2] = gamma.reshape(2, 128).T
    gbt[:, 2:4] = beta.reshape(2, 128).T

    return {
        "xt": xt.astype(np.float32), "xae": xae, "oh": oh, "scl": scl,
        "offs": offs, "ga": ga, "ge": ge, "wx": wx.astype(np.float32),
        "wpa": wpa, "wpe": wpe, "gb": gbt,
    }


def run(x, atom_idx, ele_idx, W1, b1, gamma, beta, n_cores=8, runner=None,
        num_seg=NUM_SEG):
    x = np.asarray(x, dtype=np.float32)
    atom_idx = np.asarray(atom_idx).astype(np.int64)
    ele_idx = np.asarray(ele_idx).astype(np.int64)
    W1 = np.asarray(W1, dtype=np.float32)
    gamma = np.asarray(gamma, dtype=np.float32)
    beta = np.asarray(beta, dtype=np.float32)

    n = x.shape[0]
    assert n % n_cores == 0
    rpc = n // n_cores
    assert rpc % BLK == 0
    assert np.all(np.diff(atom_idx) >= 0), "atom_idx must be sorted"

    counts = np.bincount(atom_idx, minlength=num_seg).astype(np.int64)

    # pick the largest group size (in 512-row blocks) whose segment span fits
    # in one 128-partition PSUM scatter window on every core
    def max_span(g):
        rows = BLK * g
        return max(int(atom_idx[min(i + rows, n) - 1]) - int(atom_idx[i]) + 1
                   for i in range(0, n, rows))
    gsz, w_blk = 1, None
    for g in (8, 4, 2, 1):
        if (rpc // BLK) % g:
            continue
        s = max_span(g)
        if s <= 120 or g == 1:
            gsz = g
            w_blk = max(8, ((s + 7) // 8) * 8)
            break
    assert w_blk is not None and w_blk <= 128, f"segment span too large: {w_blk}"

    in_maps = []
    for c in range(n_cores):
        in_maps.append(prep_core_inputs(x, atom_idx, ele_idx, W1, gamma, beta,
                                        rpc * c, rpc * (c + 1), w_blk, counts,
                                        num_seg, gsz))

    nc = build_program(n_cores, rpc, w_blk, num_seg, gsz)
    global LAST_NC
    LAST_NC = nc
    if runner is None:
        res = run_bass_kernel_spmd(nc, in_maps, core_ids=list(range(n_cores)))
        outs = [res.results[c]["out"] for c in range(n_cores)]
    else:
        outs = runner(nc, in_maps)

    full = np.concatenate(outs, axis=1)          # [256, n]
    return np.ascontiguousarray(full.T)          # [n, 256]


def kernel(**inputs):
    return run(inputs["x"], inputs["atom_idx"], inputs["ele_idx"],
               inputs["W1"], inputs["b1"], inputs["gamma"], inputs["beta"])
